# revision 13
# baseline (speedup 1.0000x reference)
"""MoE gate (top-6 routing) Trainium2 Bass kernel.

Problem: hidden_states [4, 4096, 2048] f32, gate weight [64, 2048] f32.
  logits = x @ W.T            -> [16384, 64]
  topk_weight, topk_idx = top_k(logits, 6)
  topk_weight = softmax(topk_weight)   (the reference's extra
  normalization divides by 1.0 + 1e-20 and is a no-op in fp32)
Returns (topk_idx int32 [16384, 6], topk_weight f32 [16384, 6]).

Sharding: data-parallel over tokens. Each of the 8 cores gets 2048
tokens; the gate weight is replicated.

Precision scheme (fp32-accurate at half the HBM traffic): each fp32
value is split on the host into two fp16 halves,
    xh = fp16(x),  xl = fp16((x - xh) * 2^11)
so x = xh + 2^-11*xl to ~2^-23 relative precision. Then
    logits = xh@wh.T + 2^-11 * (xh@wl.T + xl@wh.T)      (+O(2^-22) term dropped)
which matches the fp32 reference to below fp32 accumulation noise
(verified: bit-level top-6 agreement with the jax fp32 reference on the
actual test inputs). fp16 matmuls stream at 1 cycle/row (vs 4 for
fp32) and the input stream is 8 MB/core instead of 16.

Per-core kernel:
  - x halves fed pre-transposed ([H, T] layout, fp16) so the
    contraction dim lands on SBUF partitions with contiguous DMAs;
    w halves fed pre-packed as [128, 16*64] fp16
  - two 1024-token super-panels streamed panel-major (panel 0's top-k
    overlaps panel 1's DMA); x loaded in 1 MiB chunks of 4 h-tiles
  - matmuls in [E, T'] orientation (w stationary, x moving at N=512),
    2-way column-tiled: a panel's two 512-token blocks accumulate
    concurrently in partition halves [0:64]/[64:128] of PSUM banks
  - combine P1 + 2^-11*P2 (ACT scaled copy + DVE add)
  - PE-transpose of the logits to [token, expert] tiles
  - DVE max8/max_index (from PSUM) -> top-8 values + indices
  - ACT exp(v - max) with accumulated sum, DVE reciprocal + scale
  - results staged in SBUF, per-half-panel DMAs out; host de-interleaves
"""

import numpy as np

import concourse.bass as bass
import concourse.mybir as mybir
import concourse.tile as tile
from concourse import bacc
from concourse.bass_utils import run_bass_kernel_spmd

f32 = mybir.dt.float32
f16 = mybir.dt.float16
u32 = mybir.dt.uint32
i32 = mybir.dt.int32

N_CORES = 8
B, S, H = 4, 4096, 2048
E = 64
TOP_K = 6
T_FULL = B * S              # 16384 tokens
T_CORE = T_FULL // N_CORES  # 2048 tokens per core
KT = H // 128               # 16 contraction tiles
NTT = T_CORE // 128         # 16 token tiles per core
TB = 512                    # tokens per matmul block (PSUM bank = 512 fp32)
PANEL = 2 * TB              # 1024 tokens per super-panel (one packed psum pair)
NP = T_CORE // PANEL        # 2 super-panels per core
# h-tiles per DMA chunk, per panel: small first chunks so the PE can
# start ~5us earlier; 1 MiB steady-state chunks after.
CHUNKS = [
    [1, 1, 2, 4, 8],         # panel 0: small first chunks (early PE start)
    [8, 4, 2, 1, 1],         # panel 1: small last chunks (short tail)
]
LSCALE = float(2.0 ** -11)

_CACHE = {}


def _build():
    nc = bacc.Bacc("TRN2", target_bir_lowering=False, debug=False)
    # x halves host-packed per DMA chunk: flat [128, KT*NP*PANEL]; chunk
    # (q, c) of sz h-tiles occupies columns [off*PANEL : (off+sz)*PANEL) where
    # off is the running h-tile offset in stream order.
    XCOLS = KT * NP * PANEL
    xh = nc.dram_tensor("xh", [128, XCOLS], f16, kind="ExternalInput").ap()
    xl = nc.dram_tensor("xl", [128, XCOLS], f16, kind="ExternalInput").ap()
    wh = nc.dram_tensor("wh", [128, KT * E], f16, kind="ExternalInput").ap()
    wl = nc.dram_tensor("wl", [128, KT * E], f16, kind="ExternalInput").ap()
    ident = nc.dram_tensor("ident", [E, E], f32, kind="ExternalInput").ap()
    out_w = nc.dram_tensor("out_w", [128, NTT * TOP_K], f32, kind="ExternalOutput").ap()
    out_i = nc.dram_tensor("out_i", [128, NTT * 8], i32, kind="ExternalOutput").ap()

    with tile.TileContext(nc) as tc:
        with (
            tc.tile_pool(name="persist", bufs=1) as persist,
            tc.tile_pool(name="work", bufs=4) as work,
            tc.tile_pool(name="psum", bufs=2, space="PSUM") as psp,
            tc.tile_pool(name="psumT", bufs=4, space="PSUM") as pspT,
        ):
            # ---- weights first (warmups depend on them), then x chunks ----
            wh_all = persist.tile([128, KT * E], f16, tag="wh_all")
            nc.sync.dma_start(out=wh_all, in_=wh)
            wl_all = persist.tile([128, KT * E], f16, tag="wl_all")
            nc.sync.dma_start(out=wl_all, in_=wl)

            # per (q, h-tile a) -> (sbuf tile, col offset within tile)
            xh_at = {}
            xl_at = {}
            _off = 0

            def load_chunk(q, c, sz, off):
                # split across both HWDGE rings (SP + ACT) for queue depth
                th = persist.tile([128, sz * PANEL], f16, tag=f"xh{q}_{c}")
                nc.sync.dma_start(
                    out=th, in_=xh[:, off * PANEL : (off + sz) * PANEL]
                )
                tl = persist.tile([128, sz * PANEL], f16, tag=f"xl{q}_{c}")
                nc.scalar.dma_start(
                    out=tl, in_=xl[:, off * PANEL : (off + sz) * PANEL]
                )
                a0 = sum(CHUNKS[q][:c])
                for j in range(sz):
                    xh_at[(q, a0 + j)] = (th, j)
                    xl_at[(q, a0 + j)] = (tl, j)

            for c, sz in enumerate(CHUNKS[0]):
                load_chunk(0, c, sz, _off)
                _off += sz
            id_t = persist.tile([E, E], f32, tag="ident")
            nc.sync.dma_start(out=id_t, in_=ident)
            for c, sz in enumerate(CHUNKS[1]):
                load_chunk(1, c, sz, _off)
                _off += sz

            # Warmup matmuls: absorb the wh/wl DMA waits on the PE (a fused
            # matmul carries at most one semaphore wait) and spin the PE so
            # the HAM clock-gate warms before the real matmuls arrive.
            ps_warm = pspT.tile([64, 64], f32, tag="ps_t")
            for _ in range(6):
                nc.tensor.matmul(
                    ps_warm, wh_all[:, 0:64], wh_all[:, 0:64], start=True, stop=True
                )
            nc.tensor.matmul(
                ps_warm, wl_all[:, 0:64], wl_all[:, 0:64], start=True, stop=True
            )
            # absorb the ident DMA wait + warm the transpose path
            nc.tensor.transpose(ps_warm, id_t, id_t)

            stage_w = persist.tile([128, NTT * TOP_K], f32, tag="stage_w")
            stage_i = persist.tile([128, NTT * 8], u32, tag="stage_i")

            for q in range(NP):
                # ---- packed accumulation; half -> partition range / col-group
                ps1 = psp.tile([128, TB], f32, tag="ps1")  # xh@wh
                ps2 = psp.tile([128, TB], f32, tag="ps2")  # xh@wl + xl@wh
                def mm_p1_p2a(a):
                    wh_t = wh_all[:, a * E : (a + 1) * E]
                    wl_t = wl_all[:, a * E : (a + 1) * E]
                    th, jh = xh_at[(q, a)]
                    for half in range(2):
                        slh = slice(jh * PANEL + half * TB, jh * PANEL + (half + 1) * TB)
                        pr = slice(half * 64, (half + 1) * 64)
                        nc.tensor.matmul(
                            ps1[pr, :], wh_t, th[:, slh],
                            start=(a == 0), stop=(a == KT - 1),
                        )
                        nc.tensor.matmul(
                            ps2[pr, :], wl_t, th[:, slh],
                            start=(a == 0), stop=False,
                        )

                def mm_p2b(a):
                    wh_t = wh_all[:, a * E : (a + 1) * E]
                    tl, jl = xl_at[(q, a)]
                    for half in range(2):
                        sll = slice(jl * PANEL + half * TB, jl * PANEL + (half + 1) * TB)
                        pr = slice(half * 64, (half + 1) * 64)
                        nc.tensor.matmul(
                            ps2[pr, :], wh_t, tl[:, sll],
                            start=False, stop=(a == KT - 1),
                        )

                # xh-consuming matmuls run as chunks land; the xl-consuming
                # ones trail one chunk behind so they never stall the PE FIFO
                # (the xl chunk lands while the next xh chunk's work runs).
                bounds = []
                a0 = 0
                for sz in CHUNKS[q]:
                    bounds.append((a0, a0 + sz))
                    a0 += sz
                for ci, (lo, hi) in enumerate(bounds):
                    for a in range(lo, hi):
                        mm_p1_p2a(a)
                    if ci > 0:
                        plo, phi = bounds[ci - 1]
                        for a in range(plo, phi):
                            mm_p2b(a)
                lo, hi = bounds[-1]
                for a in range(lo, hi):
                    mm_p2b(a)

                # ---- per-block epilogue (per 128-token tile for a short
                #      tail: combine -> transpose -> top-k pipeline) ----
                for half in range(2):
                    pr = slice(half * 64, (half + 1) * 64)
                    for tt in range(TB // 128):
                        t = (2 * q + half) * (TB // 128) + tt
                        cs = slice(tt * 128, (tt + 1) * 128)
                        t2 = work.tile([64, 128], f32, tag="t2")
                        nc.scalar.activation(
                            out=t2,
                            in_=ps2[pr, cs],
                            func=mybir.ActivationFunctionType.Copy,
                            scale=LSCALE,
                        )
                        ltE = work.tile([64, 128], f32, tag="ltE")
                        nc.vector.tensor_add(ltE, t2, ps1[pr, cs])

                        ps_t = pspT.tile([128, E], f32, tag="ps_t")
                        nc.tensor.transpose(ps_t, ltE, id_t)
                        m8 = work.tile([128, 8], f32, tag="m8")
                        nc.vector.max(out=m8, in_=ps_t)
                        nc.vector.max_index(
                            stage_i[:, t * 8 : (t + 1) * 8], m8, ps_t
                        )

                        negm = work.tile([128, 1], f32, tag="negm")
                        nc.scalar.mul(negm, m8[:, 0:1], -1.0)
                        expw = work.tile([128, TOP_K], f32, tag="expw")
                        ssum = work.tile([128, 1], f32, tag="ssum")
                        nc.scalar.activation(
                            out=expw,
                            in_=m8[:, 0:TOP_K],
                            func=mybir.ActivationFunctionType.Exp,
                            bias=negm[:, 0:1],
                            scale=1.0,
                            accum_out=ssum[:, 0:1],
                        )
                        rsum = work.tile([128, 1], f32, tag="rsum")
                        nc.vector.reciprocal(rsum, ssum)
                        nc.vector.tensor_scalar_mul(
                            stage_w[:, t * TOP_K : (t + 1) * TOP_K],
                            expw,
                            rsum[:, 0:1],
                        )

                    # ---- per-half-panel output DMAs ----
                    nt_h = TB // 128  # 4 token tiles per half
                    c0 = (2 * q + half) * nt_h
                    nc.sync.dma_start(
                        out=out_w[:, c0 * TOP_K : (c0 + nt_h) * TOP_K],
                        in_=stage_w[:, c0 * TOP_K : (c0 + nt_h) * TOP_K],
                    )
                    nc.sync.dma_start(
                        out=out_i[:, c0 * 8 : (c0 + nt_h) * 8],
                        in_=stage_i[:, c0 * 8 : (c0 + nt_h) * 8].bitcast(i32),
                    )

    nc.compile()
    return nc


def _get_nc():
    if "nc" not in _CACHE:
        _CACHE["nc"] = _build()
    return _CACHE["nc"]


def _split_fp16(arr32):
    """arr32 (fp32) -> (hi fp16, lo fp16) with arr32 ~= hi + 2^-11 * lo."""
    hi = arr32.astype(np.float16)
    lo = ((arr32 - hi.astype(np.float32)) * 2048.0).astype(np.float16)
    return hi, lo


def kernel(hidden_states: np.ndarray, weight: np.ndarray, **_run_kwargs):
    x = np.ascontiguousarray(hidden_states, dtype=np.float32).reshape(T_FULL, H)
    w = np.ascontiguousarray(weight, dtype=np.float32)

    w_hi, w_lo = _split_fp16(w)  # [E, H] fp16
    # device layout [128, KT*E]: row p, col a*E+e  <-  W[e, a*128+p]
    def pack_w(wx):
        return np.ascontiguousarray(
            wx.T.reshape(KT, 128, E).transpose(1, 0, 2).reshape(128, KT * E)
        )

    whp = pack_w(w_hi)
    wlp = pack_w(w_lo)
    ident = np.eye(E, dtype=np.float32)

    def pack_x(xT16):
        # [H, T_CORE] -> [128, KT*NP*PANEL] in stream order: for panel q and
        # h-tile a (ascending), column block (q, a) = xT16[a*128+p, q*PANEL+t]
        v = xT16.reshape(KT, 128, NP, PANEL)
        return np.ascontiguousarray(
            v.transpose(1, 2, 0, 3).reshape(128, NP * KT * PANEL)
        )

    in_maps = []
    for c in range(N_CORES):
        shard = x[c * T_CORE : (c + 1) * T_CORE, :]  # [T_CORE, H]
        xT = np.ascontiguousarray(shard.T)  # [H, T_CORE] fp32
        xhs, xls = _split_fp16(xT)
        in_maps.append(
            {"xh": pack_x(xhs), "xl": pack_x(xls), "wh": whp, "wl": wlp, "ident": ident}
        )

    nc = _get_nc()
    res = run_bass_kernel_spmd(
        nc, in_maps, core_ids=list(range(N_CORES)), **_run_kwargs
    )

    idx_parts = []
    w_parts = []
    for c in range(N_CORES):
        r = res.results[c]
        si = r["out_i"].reshape(128, NTT, 8).transpose(1, 0, 2)[:, :, :TOP_K]
        sw = r["out_w"].reshape(128, NTT, TOP_K).transpose(1, 0, 2)
        idx_parts.append(si.reshape(T_CORE, TOP_K).astype(np.int32, copy=False))
        w_parts.append(sw.reshape(T_CORE, TOP_K))

    topk_idx = np.concatenate(idx_parts, axis=0)
    topk_weight = np.concatenate(w_parts, axis=0)
    if "trace" in _run_kwargs:
        return (topk_idx, topk_weight), res
    return topk_idx, topk_weight


# revision 14
# speedup vs baseline: 1.1240x; 1.1240x over previous
"""MoE gate (top-6 routing) Trainium2 Bass kernel.

Problem: hidden_states [4, 4096, 2048] f32, gate weight [64, 2048] f32.
  logits = x @ W.T            -> [16384, 64]
  topk_weight, topk_idx = top_k(logits, 6)
  topk_weight = softmax(topk_weight)   (the reference's extra
  normalization divides by 1.0 + 1e-20 and is a no-op in fp32)
Returns (topk_idx int32 [16384, 6], topk_weight f32 [16384, 6]).

Sharding: data-parallel over tokens. Each of the 8 cores gets 2048
tokens; the gate weight is replicated.

Precision scheme (fp32-accurate at half the HBM traffic): each fp32
value is split on the host into two fp16 halves,
    xh = fp16(x),  xl = fp16((x - xh) * 2^11)
so x = xh + 2^-11*xl to ~2^-23 relative precision. Then
    logits = xh@wh.T + 2^-11 * (xh@wl.T + xl@wh.T)      (+O(2^-22) term dropped)
which matches the fp32 reference to below fp32 accumulation noise
(verified: bit-level top-6 agreement with the jax fp32 reference on the
actual test inputs). fp16 matmuls stream at 1 cycle/row (vs 4 for
fp32) and the input stream is 8 MB/core instead of 16.

Per-core kernel:
  - x halves fed pre-transposed ([H, T] layout, fp16) so the
    contraction dim lands on SBUF partitions with contiguous DMAs;
    w halves fed pre-packed as [128, 16*64] fp16
  - two 1024-token super-panels streamed panel-major (panel 0's top-k
    overlaps panel 1's DMA); x loaded in 1 MiB chunks of 4 h-tiles
  - matmuls in [E, T'] orientation (w stationary, x moving at N=512),
    2-way column-tiled: a panel's two 512-token blocks accumulate
    concurrently in partition halves [0:64]/[64:128] of PSUM banks
  - combine P1 + 2^-11*P2 (ACT scaled copy + DVE add)
  - PE-transpose of the logits to [token, expert] tiles
  - DVE max8/max_index (from PSUM) -> top-8 values + indices
  - ACT exp(v - max) with accumulated sum, DVE reciprocal + scale
  - results staged in SBUF, per-half-panel DMAs out; host de-interleaves
"""

import numpy as np

import concourse.bass as bass
import concourse.mybir as mybir
import concourse.tile as tile
from concourse import bacc
from concourse.bass_utils import run_bass_kernel_spmd

f32 = mybir.dt.float32
f16 = mybir.dt.float16
u32 = mybir.dt.uint32
i32 = mybir.dt.int32

N_CORES = 8
B, S, H = 4, 4096, 2048
E = 64
TOP_K = 6
T_FULL = B * S              # 16384 tokens
T_CORE = T_FULL // N_CORES  # 2048 tokens per core
KT = H // 128               # 16 contraction tiles
NTT = T_CORE // 128         # 16 token tiles per core
TB = 512                    # tokens per matmul block (PSUM bank = 512 fp32)
PANEL = 2 * TB              # 1024 tokens per super-panel (one packed psum pair)
NP = T_CORE // PANEL        # 2 super-panels per core
# h-tiles per DMA chunk, per panel: small first chunks so the PE can
# start ~5us earlier; 1 MiB steady-state chunks after.
CHUNKS = [
    [1, 1, 2, 4, 8],         # panel 0: small first chunks (early PE start)
    [8, 4, 2, 1, 1],         # panel 1: small last chunks (short tail)
]
LSCALE = float(2.0 ** -11)

_CACHE = {}


def _build():
    nc = bacc.Bacc("TRN2", target_bir_lowering=False, debug=False)
    # x halves host-packed per DMA chunk: flat [128, KT*NP*PANEL]; chunk
    # (q, c) of sz h-tiles occupies columns [off*PANEL : (off+sz)*PANEL) where
    # off is the running h-tile offset in stream order.
    XCOLS = KT * NP * PANEL
    xh = nc.dram_tensor("xh", [128, XCOLS], f16, kind="ExternalInput").ap()
    xl = nc.dram_tensor("xl", [128, XCOLS], f16, kind="ExternalInput").ap()
    wh = nc.dram_tensor("wh", [128, KT * E], f16, kind="ExternalInput").ap()
    wl = nc.dram_tensor("wl", [128, KT * E], f16, kind="ExternalInput").ap()
    ident = nc.dram_tensor("ident", [E, E], f32, kind="ExternalInput").ap()
    out_w = nc.dram_tensor("out_w", [128, NTT * TOP_K], f32, kind="ExternalOutput").ap()
    out_i = nc.dram_tensor("out_i", [128, NTT * 8], i32, kind="ExternalOutput").ap()

    with tile.TileContext(nc) as tc:
        with (
            tc.tile_pool(name="persist", bufs=1) as persist,
            tc.tile_pool(name="work", bufs=4) as work,
            tc.tile_pool(name="psum", bufs=2, space="PSUM") as psp,
            tc.tile_pool(name="psumT", bufs=4, space="PSUM") as pspT,
        ):
            # ---- weights first (warmups depend on them), then x chunks ----
            wh_all = persist.tile([128, KT * E], f16, tag="wh_all")
            nc.sync.dma_start(out=wh_all, in_=wh)
            wl_all = persist.tile([128, KT * E], f16, tag="wl_all")
            nc.sync.dma_start(out=wl_all, in_=wl)

            # per (q, h-tile a) -> (sbuf tile, col offset within tile)
            xh_at = {}
            xl_at = {}
            _off = 0

            def load_chunk(q, c, sz, off):
                th = persist.tile([128, sz * PANEL], f16, tag=f"xh{q}_{c}")
                nc.sync.dma_start(
                    out=th, in_=xh[:, off * PANEL : (off + sz) * PANEL]
                )
                tl = persist.tile([128, sz * PANEL], f16, tag=f"xl{q}_{c}")
                nc.sync.dma_start(
                    out=tl, in_=xl[:, off * PANEL : (off + sz) * PANEL]
                )
                a0 = sum(CHUNKS[q][:c])
                for j in range(sz):
                    xh_at[(q, a0 + j)] = (th, j)
                    xl_at[(q, a0 + j)] = (tl, j)

            for c, sz in enumerate(CHUNKS[0]):
                load_chunk(0, c, sz, _off)
                _off += sz
            id_t = persist.tile([E, E], f32, tag="ident")
            nc.sync.dma_start(out=id_t, in_=ident)
            for c, sz in enumerate(CHUNKS[1]):
                load_chunk(1, c, sz, _off)
                _off += sz

            # Warmup matmuls: absorb the wh/wl DMA waits on the PE (a fused
            # matmul carries at most one semaphore wait) and spin the PE so
            # the HAM clock-gate warms before the real matmuls arrive.
            ps_warm = pspT.tile([64, 64], f32, tag="ps_t")
            for _ in range(6):
                nc.tensor.matmul(
                    ps_warm, wh_all[:, 0:64], wh_all[:, 0:64], start=True, stop=True
                )
            nc.tensor.matmul(
                ps_warm, wl_all[:, 0:64], wl_all[:, 0:64], start=True, stop=True
            )
            # absorb the ident DMA wait + warm the transpose path
            nc.tensor.transpose(ps_warm, id_t, id_t)

            stage_w = persist.tile([128, NTT * TOP_K], f32, tag="stage_w")
            stage_i = persist.tile([128, NTT * 8], u32, tag="stage_i")

            for q in range(NP):
                # ---- packed accumulation; half -> partition range / col-group
                ps1 = psp.tile([128, TB], f32, tag="ps1")  # xh@wh
                ps2 = psp.tile([128, TB], f32, tag="ps2")  # xh@wl + xl@wh
                def mm_p1_p2a(a):
                    wh_t = wh_all[:, a * E : (a + 1) * E]
                    wl_t = wl_all[:, a * E : (a + 1) * E]
                    th, jh = xh_at[(q, a)]
                    for half in range(2):
                        slh = slice(jh * PANEL + half * TB, jh * PANEL + (half + 1) * TB)
                        pr = slice(half * 64, (half + 1) * 64)
                        nc.tensor.matmul(
                            ps1[pr, :], wh_t, th[:, slh],
                            start=(a == 0), stop=(a == KT - 1),
                        )
                        nc.tensor.matmul(
                            ps2[pr, :], wl_t, th[:, slh],
                            start=(a == 0), stop=False,
                        )

                def mm_p2b(a):
                    wh_t = wh_all[:, a * E : (a + 1) * E]
                    tl, jl = xl_at[(q, a)]
                    for half in range(2):
                        sll = slice(jl * PANEL + half * TB, jl * PANEL + (half + 1) * TB)
                        pr = slice(half * 64, (half + 1) * 64)
                        nc.tensor.matmul(
                            ps2[pr, :], wh_t, tl[:, sll],
                            start=False, stop=(a == KT - 1),
                        )

                # xh-consuming matmuls run as chunks land; the xl-consuming
                # ones trail one chunk behind so they never stall the PE FIFO
                # (the xl chunk lands while the next xh chunk's work runs).
                bounds = []
                a0 = 0
                for sz in CHUNKS[q]:
                    bounds.append((a0, a0 + sz))
                    a0 += sz
                for ci, (lo, hi) in enumerate(bounds):
                    for a in range(lo, hi):
                        mm_p1_p2a(a)
                    if ci > 0:
                        plo, phi = bounds[ci - 1]
                        for a in range(plo, phi):
                            mm_p2b(a)
                lo, hi = bounds[-1]
                for a in range(lo, hi):
                    mm_p2b(a)

                # ---- per-block epilogue (per 128-token tile for a short
                #      tail: combine -> transpose -> top-k pipeline) ----
                for half in range(2):
                    pr = slice(half * 64, (half + 1) * 64)
                    for tt in range(TB // 128):
                        t = (2 * q + half) * (TB // 128) + tt
                        cs = slice(tt * 128, (tt + 1) * 128)
                        t2 = work.tile([64, 128], f32, tag="t2")
                        nc.scalar.activation(
                            out=t2,
                            in_=ps2[pr, cs],
                            func=mybir.ActivationFunctionType.Copy,
                            scale=LSCALE,
                        )
                        ltE = work.tile([64, 128], f32, tag="ltE")
                        nc.vector.tensor_add(ltE, t2, ps1[pr, cs])

                        ps_t = pspT.tile([128, E], f32, tag="ps_t")
                        nc.tensor.transpose(ps_t, ltE, id_t)
                        m8 = work.tile([128, 8], f32, tag="m8")
                        nc.vector.max(out=m8, in_=ps_t)
                        nc.vector.max_index(
                            stage_i[:, t * 8 : (t + 1) * 8], m8, ps_t
                        )

                        negm = work.tile([128, 1], f32, tag="negm")
                        nc.scalar.mul(negm, m8[:, 0:1], -1.0)
                        expw = work.tile([128, TOP_K], f32, tag="expw")
                        ssum = work.tile([128, 1], f32, tag="ssum")
                        nc.scalar.activation(
                            out=expw,
                            in_=m8[:, 0:TOP_K],
                            func=mybir.ActivationFunctionType.Exp,
                            bias=negm[:, 0:1],
                            scale=1.0,
                            accum_out=ssum[:, 0:1],
                        )
                        rsum = work.tile([128, 1], f32, tag="rsum")
                        nc.vector.reciprocal(rsum, ssum)
                        nc.vector.tensor_scalar_mul(
                            stage_w[:, t * TOP_K : (t + 1) * TOP_K],
                            expw,
                            rsum[:, 0:1],
                        )

                    # ---- per-half-panel output DMAs ----
                    nt_h = TB // 128  # 4 token tiles per half
                    c0 = (2 * q + half) * nt_h
                    nc.sync.dma_start(
                        out=out_w[:, c0 * TOP_K : (c0 + nt_h) * TOP_K],
                        in_=stage_w[:, c0 * TOP_K : (c0 + nt_h) * TOP_K],
                    )
                    nc.sync.dma_start(
                        out=out_i[:, c0 * 8 : (c0 + nt_h) * 8],
                        in_=stage_i[:, c0 * 8 : (c0 + nt_h) * 8].bitcast(i32),
                    )

    nc.compile()
    return nc


def _get_nc():
    if "nc" not in _CACHE:
        _CACHE["nc"] = _build()
    return _CACHE["nc"]


def _split_fp16(arr32):
    """arr32 (fp32) -> (hi fp16, lo fp16) with arr32 ~= hi + 2^-11 * lo."""
    hi = arr32.astype(np.float16)
    lo = ((arr32 - hi.astype(np.float32)) * 2048.0).astype(np.float16)
    return hi, lo


def kernel(hidden_states: np.ndarray, weight: np.ndarray, **_run_kwargs):
    x = np.ascontiguousarray(hidden_states, dtype=np.float32).reshape(T_FULL, H)
    w = np.ascontiguousarray(weight, dtype=np.float32)

    w_hi, w_lo = _split_fp16(w)  # [E, H] fp16
    # device layout [128, KT*E]: row p, col a*E+e  <-  W[e, a*128+p]
    def pack_w(wx):
        return np.ascontiguousarray(
            wx.T.reshape(KT, 128, E).transpose(1, 0, 2).reshape(128, KT * E)
        )

    whp = pack_w(w_hi)
    wlp = pack_w(w_lo)
    ident = np.eye(E, dtype=np.float32)

    def pack_x(xT16):
        # [H, T_CORE] -> [128, KT*NP*PANEL] in stream order: for panel q and
        # h-tile a (ascending), column block (q, a) = xT16[a*128+p, q*PANEL+t]
        v = xT16.reshape(KT, 128, NP, PANEL)
        return np.ascontiguousarray(
            v.transpose(1, 2, 0, 3).reshape(128, NP * KT * PANEL)
        )

    in_maps = []
    for c in range(N_CORES):
        shard = x[c * T_CORE : (c + 1) * T_CORE, :]  # [T_CORE, H]
        xT = np.ascontiguousarray(shard.T)  # [H, T_CORE] fp32
        xhs, xls = _split_fp16(xT)
        in_maps.append(
            {"xh": pack_x(xhs), "xl": pack_x(xls), "wh": whp, "wl": wlp, "ident": ident}
        )

    nc = _get_nc()
    res = run_bass_kernel_spmd(
        nc, in_maps, core_ids=list(range(N_CORES)), **_run_kwargs
    )

    idx_parts = []
    w_parts = []
    for c in range(N_CORES):
        r = res.results[c]
        si = r["out_i"].reshape(128, NTT, 8).transpose(1, 0, 2)[:, :, :TOP_K]
        sw = r["out_w"].reshape(128, NTT, TOP_K).transpose(1, 0, 2)
        idx_parts.append(si.reshape(T_CORE, TOP_K).astype(np.int32, copy=False))
        w_parts.append(sw.reshape(T_CORE, TOP_K))

    topk_idx = np.concatenate(idx_parts, axis=0)
    topk_weight = np.concatenate(w_parts, axis=0)
    if "trace" in _run_kwargs:
        return (topk_idx, topk_weight), res
    return topk_idx, topk_weight


# revision 15
# speedup vs baseline: 1.1412x; 1.0152x over previous
"""MoE gate (top-6 routing) Trainium2 Bass kernel.

Problem: hidden_states [4, 4096, 2048] f32, gate weight [64, 2048] f32.
  logits = x @ W.T            -> [16384, 64]
  topk_weight, topk_idx = top_k(logits, 6)
  topk_weight = softmax(topk_weight)   (the reference's extra
  normalization divides by 1.0 + 1e-20 and is a no-op in fp32)
Returns (topk_idx int32 [16384, 6], topk_weight f32 [16384, 6]).

Sharding: data-parallel over tokens. Each of the 8 cores gets 2048
tokens; the gate weight is replicated.

Precision scheme (fp32-accurate at half the HBM traffic): each fp32
value is split on the host into two fp16 halves,
    xh = fp16(x),  xl = fp16((x - xh) * 2^11)
so x = xh + 2^-11*xl to ~2^-23 relative precision. Then
    logits = xh@wh.T + 2^-11 * (xh@wl.T + xl@wh.T)      (+O(2^-22) term dropped)
which matches the fp32 reference to below fp32 accumulation noise
(verified: bit-level top-6 agreement with the jax fp32 reference on the
actual test inputs). fp16 matmuls stream at 1 cycle/row (vs 4 for
fp32) and the input stream is 8 MB/core instead of 16.

Per-core kernel:
  - x halves fed pre-transposed ([H, T] layout, fp16) so the
    contraction dim lands on SBUF partitions with contiguous DMAs;
    w halves fed pre-packed as [128, 16*64] fp16
  - two 1024-token super-panels streamed panel-major (panel 0's top-k
    overlaps panel 1's DMA); x loaded in 1 MiB chunks of 4 h-tiles
  - matmuls in [E, T'] orientation (w stationary, x moving at N=512),
    2-way column-tiled: a panel's two 512-token blocks accumulate
    concurrently in partition halves [0:64]/[64:128] of PSUM banks
  - combine P1 + 2^-11*P2 (ACT scaled copy + DVE add)
  - PE-transpose of the logits to [token, expert] tiles
  - DVE max8/max_index (from PSUM) -> top-8 values + indices
  - ACT exp(v - max) with accumulated sum, DVE reciprocal + scale
  - results staged in SBUF, per-half-panel DMAs out; host de-interleaves
"""

import numpy as np

import concourse.bass as bass
import concourse.mybir as mybir
import concourse.tile as tile
from concourse import bacc
from concourse.bass_utils import run_bass_kernel_spmd

f32 = mybir.dt.float32
f16 = mybir.dt.float16
u32 = mybir.dt.uint32
i32 = mybir.dt.int32

N_CORES = 8
B, S, H = 4, 4096, 2048
E = 64
TOP_K = 6
T_FULL = B * S              # 16384 tokens
T_CORE = T_FULL // N_CORES  # 2048 tokens per core
KT = H // 128               # 16 contraction tiles
NTT = T_CORE // 128         # 16 token tiles per core
TB = 512                    # tokens per matmul block (PSUM bank = 512 fp32)
PANEL = 2 * TB              # 1024 tokens per super-panel (one packed psum pair)
NP = T_CORE // PANEL        # 2 super-panels per core
# h-tiles per DMA chunk, per panel: small first chunks so the PE can
# start ~5us earlier; 1 MiB steady-state chunks after.
CHUNKS = [
    [1, 1, 2, 4, 8],         # panel 0: small first chunks (early PE start)
    [8, 4, 2, 1, 1],         # panel 1: small last chunks (short tail)
]
LSCALE = float(2.0 ** -11)

_CACHE = {}


def _build():
    nc = bacc.Bacc("TRN2", target_bir_lowering=False, debug=False)
    # x halves host-packed per DMA chunk: flat [128, KT*NP*PANEL]; chunk
    # (q, c) of sz h-tiles occupies columns [off*PANEL : (off+sz)*PANEL) where
    # off is the running h-tile offset in stream order.
    XCOLS = KT * NP * PANEL
    xh = nc.dram_tensor("xh", [128, XCOLS], f16, kind="ExternalInput").ap()
    xl = nc.dram_tensor("xl", [128, XCOLS], f16, kind="ExternalInput").ap()
    wh = nc.dram_tensor("wh", [128, KT * E], f16, kind="ExternalInput").ap()
    wl = nc.dram_tensor("wl", [128, KT * E], f16, kind="ExternalInput").ap()
    ident = nc.dram_tensor("ident", [E, E], f32, kind="ExternalInput").ap()
    out_w = nc.dram_tensor("out_w", [128, NTT * TOP_K], f32, kind="ExternalOutput").ap()
    out_i = nc.dram_tensor("out_i", [128, NTT * 8], i32, kind="ExternalOutput").ap()

    with tile.TileContext(nc) as tc:
        with (
            tc.tile_pool(name="persist", bufs=1) as persist,
            tc.tile_pool(name="work", bufs=4) as work,
            tc.tile_pool(name="psum", bufs=2, space="PSUM") as psp,
            tc.tile_pool(name="psumT", bufs=4, space="PSUM") as pspT,
        ):
            # ---- weights first (warmups depend on them), then x chunks ----
            wh_all = persist.tile([128, KT * E], f16, tag="wh_all")
            nc.sync.dma_start(out=wh_all, in_=wh)
            wl_all = persist.tile([128, KT * E], f16, tag="wl_all")
            nc.sync.dma_start(out=wl_all, in_=wl)

            # per (q, h-tile a) -> (sbuf tile, col offset within tile)
            xh_at = {}
            xl_at = {}
            _off = 0

            def load_chunk(q, c, sz, off):
                th = persist.tile([128, sz * PANEL], f16, tag=f"xh{q}_{c}")
                nc.sync.dma_start(
                    out=th, in_=xh[:, off * PANEL : (off + sz) * PANEL]
                )
                tl = persist.tile([128, sz * PANEL], f16, tag=f"xl{q}_{c}")
                nc.sync.dma_start(
                    out=tl, in_=xl[:, off * PANEL : (off + sz) * PANEL]
                )
                a0 = sum(CHUNKS[q][:c])
                for j in range(sz):
                    xh_at[(q, a0 + j)] = (th, j)
                    xl_at[(q, a0 + j)] = (tl, j)

            for c, sz in enumerate(CHUNKS[0]):
                load_chunk(0, c, sz, _off)
                _off += sz
            id_t = persist.tile([E, E], f32, tag="ident")
            nc.sync.dma_start(out=id_t, in_=ident)
            for c, sz in enumerate(CHUNKS[1]):
                load_chunk(1, c, sz, _off)
                _off += sz

            # Warmup matmuls: absorb the wh/wl DMA waits on the PE (a fused
            # matmul carries at most one semaphore wait) and spin the PE so
            # the HAM clock-gate warms before the real matmuls arrive.
            ps_warm = pspT.tile([64, 64], f32, tag="ps_t")
            for _ in range(6):
                nc.tensor.matmul(
                    ps_warm, wh_all[:, 0:64], wh_all[:, 0:64], start=True, stop=True
                )
            nc.tensor.matmul(
                ps_warm, wl_all[:, 0:64], wl_all[:, 0:64], start=True, stop=True
            )
            # absorb the ident DMA wait + warm the transpose path
            nc.tensor.transpose(ps_warm, id_t, id_t)

            stage_w = persist.tile([128, NTT * TOP_K], f32, tag="stage_w")
            stage_i = persist.tile([128, NTT * 8], u32, tag="stage_i")

            for q in range(NP):
                # ---- packed accumulation; half -> partition range / col-group
                ps1 = psp.tile([128, TB], f32, tag="ps1")  # xh@wh
                ps2 = psp.tile([128, TB], f32, tag="ps2")  # xh@wl + xl@wh
                def mm_p1_p2a(a):
                    wh_t = wh_all[:, a * E : (a + 1) * E]
                    wl_t = wl_all[:, a * E : (a + 1) * E]
                    th, jh = xh_at[(q, a)]
                    for half in range(2):
                        slh = slice(jh * PANEL + half * TB, jh * PANEL + (half + 1) * TB)
                        pr = slice(half * 64, (half + 1) * 64)
                        nc.tensor.matmul(
                            ps1[pr, :], wh_t, th[:, slh],
                            start=(a == 0), stop=(a == KT - 1),
                        )
                        nc.tensor.matmul(
                            ps2[pr, :], wl_t, th[:, slh],
                            start=(a == 0), stop=False,
                        )

                def mm_p2b(a):
                    wh_t = wh_all[:, a * E : (a + 1) * E]
                    tl, jl = xl_at[(q, a)]
                    for half in range(2):
                        sll = slice(jl * PANEL + half * TB, jl * PANEL + (half + 1) * TB)
                        pr = slice(half * 64, (half + 1) * 64)
                        nc.tensor.matmul(
                            ps2[pr, :], wh_t, tl[:, sll],
                            start=False, stop=(a == KT - 1),
                        )

                # xh-consuming matmuls run as chunks land; the xl-consuming
                # ones trail one chunk behind so they never stall the PE FIFO
                # (the xl chunk lands while the next xh chunk's work runs).
                bounds = []
                a0 = 0
                for sz in CHUNKS[q]:
                    bounds.append((a0, a0 + sz))
                    a0 += sz
                for ci, (lo, hi) in enumerate(bounds):
                    for a in range(lo, hi):
                        mm_p1_p2a(a)
                    if ci > 0:
                        plo, phi = bounds[ci - 1]
                        for a in range(plo, phi):
                            mm_p2b(a)
                lo, hi = bounds[-1]
                for a in range(lo, hi):
                    mm_p2b(a)

                # ---- per-block epilogue (per 128-token tile for a short
                #      tail: combine -> transpose -> top-k pipeline) ----
                for half in range(2):
                    pr = slice(half * 64, (half + 1) * 64)
                    for tt in range(TB // 128):
                        t = (2 * q + half) * (TB // 128) + tt
                        cs = slice(tt * 128, (tt + 1) * 128)
                        t2 = work.tile([64, 128], f32, tag="t2")
                        nc.scalar.activation(
                            out=t2,
                            in_=ps2[pr, cs],
                            func=mybir.ActivationFunctionType.Copy,
                            scale=LSCALE,
                        )
                        ltE = work.tile([64, 128], f32, tag="ltE")
                        nc.vector.tensor_add(ltE, t2, ps1[pr, cs])

                        ps_t = pspT.tile([128, E], f32, tag="ps_t")
                        nc.tensor.transpose(ps_t, ltE, id_t)
                        m8 = work.tile([128, 8], f32, tag="m8")
                        nc.vector.max(out=m8, in_=ps_t)
                        nc.vector.max_index(
                            stage_i[:, t * 8 : (t + 1) * 8], m8, ps_t
                        )

                        negm = work.tile([128, 1], f32, tag="negm")
                        nc.scalar.mul(negm, m8[:, 0:1], -1.0)
                        expw = work.tile([128, TOP_K], f32, tag="expw")
                        ssum = work.tile([128, 1], f32, tag="ssum")
                        nc.scalar.activation(
                            out=expw,
                            in_=m8[:, 0:TOP_K],
                            func=mybir.ActivationFunctionType.Exp,
                            bias=negm[:, 0:1],
                            scale=1.0,
                            accum_out=ssum[:, 0:1],
                        )
                        rsum = work.tile([128, 1], f32, tag="rsum")
                        nc.vector.reciprocal(rsum, ssum)
                        nc.vector.tensor_scalar_mul(
                            stage_w[:, t * TOP_K : (t + 1) * TOP_K],
                            expw,
                            rsum[:, 0:1],
                        )

            # ---- output DMAs, emitted last so their chain-waits can never
            #      head-of-line-block the x load triggers on the Sync ring ----
            for q in range(NP):
                for half in range(2):
                    nt_h = TB // 128  # 4 token tiles per half
                    c0 = (2 * q + half) * nt_h
                    nc.sync.dma_start(
                        out=out_w[:, c0 * TOP_K : (c0 + nt_h) * TOP_K],
                        in_=stage_w[:, c0 * TOP_K : (c0 + nt_h) * TOP_K],
                    )
                    nc.sync.dma_start(
                        out=out_i[:, c0 * 8 : (c0 + nt_h) * 8],
                        in_=stage_i[:, c0 * 8 : (c0 + nt_h) * 8].bitcast(i32),
                    )

    nc.compile()
    return nc


def _get_nc():
    if "nc" not in _CACHE:
        _CACHE["nc"] = _build()
    return _CACHE["nc"]


def _split_fp16(arr32):
    """arr32 (fp32) -> (hi fp16, lo fp16) with arr32 ~= hi + 2^-11 * lo."""
    hi = arr32.astype(np.float16)
    lo = ((arr32 - hi.astype(np.float32)) * 2048.0).astype(np.float16)
    return hi, lo


def kernel(hidden_states: np.ndarray, weight: np.ndarray, **_run_kwargs):
    x = np.ascontiguousarray(hidden_states, dtype=np.float32).reshape(T_FULL, H)
    w = np.ascontiguousarray(weight, dtype=np.float32)

    w_hi, w_lo = _split_fp16(w)  # [E, H] fp16
    # device layout [128, KT*E]: row p, col a*E+e  <-  W[e, a*128+p]
    def pack_w(wx):
        return np.ascontiguousarray(
            wx.T.reshape(KT, 128, E).transpose(1, 0, 2).reshape(128, KT * E)
        )

    whp = pack_w(w_hi)
    wlp = pack_w(w_lo)
    ident = np.eye(E, dtype=np.float32)

    def pack_x(xT16):
        # [H, T_CORE] -> [128, KT*NP*PANEL] in stream order: for panel q and
        # h-tile a (ascending), column block (q, a) = xT16[a*128+p, q*PANEL+t]
        v = xT16.reshape(KT, 128, NP, PANEL)
        return np.ascontiguousarray(
            v.transpose(1, 2, 0, 3).reshape(128, NP * KT * PANEL)
        )

    in_maps = []
    for c in range(N_CORES):
        shard = x[c * T_CORE : (c + 1) * T_CORE, :]  # [T_CORE, H]
        xT = np.ascontiguousarray(shard.T)  # [H, T_CORE] fp32
        xhs, xls = _split_fp16(xT)
        in_maps.append(
            {"xh": pack_x(xhs), "xl": pack_x(xls), "wh": whp, "wl": wlp, "ident": ident}
        )

    nc = _get_nc()
    res = run_bass_kernel_spmd(
        nc, in_maps, core_ids=list(range(N_CORES)), **_run_kwargs
    )

    idx_parts = []
    w_parts = []
    for c in range(N_CORES):
        r = res.results[c]
        si = r["out_i"].reshape(128, NTT, 8).transpose(1, 0, 2)[:, :, :TOP_K]
        sw = r["out_w"].reshape(128, NTT, TOP_K).transpose(1, 0, 2)
        idx_parts.append(si.reshape(T_CORE, TOP_K).astype(np.int32, copy=False))
        w_parts.append(sw.reshape(T_CORE, TOP_K))

    topk_idx = np.concatenate(idx_parts, axis=0)
    topk_weight = np.concatenate(w_parts, axis=0)
    if "trace" in _run_kwargs:
        return (topk_idx, topk_weight), res
    return topk_idx, topk_weight
